# revision 1
# baseline (speedup 1.0000x reference)
"""ContextualConv1d Trainium2 kernel (polyphase scheme).

Problem: grouped conv1d (N=32, C_in=256, L=4096, C_out=256, K=9, groups=4,
pad=4) + broadcast context term c @ c_weight.T + bias.

Sharding: data-parallel over batch N across 8 cores (4 batches/core).

Conv strategy — polyphase decomposition for full PE utilization:
  x is split host-side into even/odd phases. For one group, the matmul
  contraction packs (64 channels x 2 input phases) = 128 rows, and the
  output partitions pack (64 out channels x 2 output parities) = 128.
  The K=9 conv then becomes 5 accumulating float32r matmuls (phase shifts
  s=0..4) with dense 128x128 stationary operands (~90% MAC utilization,
  vs 50% for the naive group-pair block-diagonal form):

    ps[(o,d), m] += lhsT_s[(i,ph), (o,d)] * x2[(i,ph), m+s]
    lhsT_s[(i,0),(o,0)] = W[o,i,2s]    lhsT_s[(i,1),(o,0)] = W[o,i,2s+1]
    lhsT_s[(i,0),(o,1)] = W[o,i,2s-1]  lhsT_s[(i,1),(o,1)] = W[o,i,2s]
    (out-of-range taps are zero blocks)

  y[o, 2m+d] = ps[(o,d), m]. The phase split of x and the parity merge of
  the output are free host-side numpy reshapes (done during shard/unshard).

Precision/perf choices (all measured on HW via paired repeat-loop timing;
the terminal is shared, so decisions use per-round paired comparisons and
the 30th-percentile-of-diffs estimator against one-sided contention noise):
  - Conv inputs in fp16 (CONV_DTYPE): half the x-load DMA bytes, and
    measurably FASTER than float32r on the PE (pure-MM microbench ~53 vs
    ~68 us/iter for 320 MMs — fp16 moving operands stream >1 col/cycle).
    fp8 was ruled out by accuracy: any e4m3 operand quantization alone
    costs ~2.5e-2 max rel err vs the 2e-2 gate.
  - Output stored fp16 (OUT_FP16), upcast to f32 on host: halves the
    out-store DMA. End-to-end max rel err 5.2e-4 vs the f32 reference.
  - ORDER="sgt": within one (n,g) row, shift-outer / tile-inner so each
    128x128 stationary weight serves 4 back-to-back MMs into 4 PSUM banks
    (LDWEIGHTS amortized 4x; beat tile-outer by ~7-18 us/iter in paired
    rounds). 8 PSUM banks = 2 rows in flight.
  - DRAIN="split" + DRAIN_WIDTH=2: PSUM->SBUF drains (fp16 cast +
    per-partition ctx+bias add) alternate DVE / ACT per [128,1024]
    two-bank op, so neither engine serializes the drain stream.
  - DMA_RINGS="sp_act": both hardware DGE rings (SP+ACT) carry the
    525KB x loads / 512KB out stores, alternating per row; XBUFS=12 rows
    of x prefetch. The gpsimd SWDGE path measured slower.
Roofline position: PE streaming floor ~35-50 us/core (fp16 rate), DMA
floor ~28-47 us/core (16.8 MB/iter at ~360-600 GB/s); measured quiet-mode
~62-70 us/iter steady state, ~76+ us under terminal load.
"""

import sys

if "/opt/trn_rl_repo" not in sys.path:
    sys.path.insert(0, "/opt/trn_rl_repo")

import numpy as np

N, C_IN, L = 32, 256, 4096
C_OUT, K, GROUPS = 256, 9, 4
C_DIM, PAD = 128, 4
NCORES = 8
NB = N // NCORES          # batches per core
M = L // 2                # output phase positions (2048)
MPAD = (L + 2 * PAD) // 2  # padded phase length (2052)
MT = 512                  # phase cols per PSUM tile (one bank of fp32)
NMT = M // MT             # 4 L-tiles per (n, g)
NSHIFT = 5                # phase shifts (= ceil(K/2))
HALO = NSHIFT - 1

# Extra kwargs for run_bass_kernel_spmd (e.g. trace=True) set by a harness;
# the BassKernelResults lands in LAST_RESULT.
RUN_KWARGS: dict = {}
LAST_RESULT = None

_prog_cache: dict = {}

# Matmul input dtype for the conv path: "f32r" (safe, ~1.4e-4 rel err) or
# "fp16" (~2x less x-load DMA, ~2.8e-4 rel err).
CONV_DTYPE = "fp16"
# Store the output phase tensor as fp16 (halves out-DMA; host upcasts to
# f32; adds ~2.4e-4 output quantization).
OUT_FP16 = True
# One DMA per (n, g) row (525 KB load / 512 KB store) instead of 4 smaller
# tile DMAs: bigger lines, fewer descriptors.
BIG_TILES = True
# PSUM->SBUF drain engine: "dve" (vector only) or "split" (alternate
# DVE / ACT so neither engine is the serial bottleneck).
DRAIN = "split"
# Matmul loop order within one (n, g) row: "tgs" = tile-outer (stationary
# weight changes every MM), "sgt" = shift-outer (each stationary serves the
# 4 tile MMs back-to-back, amortizing LDWEIGHTS 4x; needs 4 PSUM banks live).
ORDER = "sgt"
# Output stores per (n, g) row (sgt path): 1 = one 512KB store, k = k chunks
# so chunk stores overlap later drains.
STORE_CHUNKS = 1
# DMA issue queues: "sp_pool" = SP + gpsimd(SWDGE), "sp_act" = the two fast
# HWDGE rings (ACT then carries both drains and half the DMA issuance).
DMA_RINGS = "sp_act"
# x-tile prefetch depth (SBUF bufs of 525KB each).
XBUFS = 12
# PSUM tiles per drain op (sgt path): 1 = [128,512] drains, 2/4 = contiguous
# multi-bank PSUM tiles drained in one wider DVE/ACT op (less per-op
# overhead; in sgt all banks of a row finish within ~1 MM of each other).
DRAIN_WIDTH = 2


def _build_program(repeats: int = 1, conv_dtype: str | None = None,
                   out_fp16: bool | None = None, big_tiles: bool | None = None,
                   drain: str | None = None, order: str | None = None,
                   store_chunks: int | None = None, dma_rings: str | None = None,
                   xbufs: int | None = None, drain_width: int | None = None):
    import concourse.bacc as bacc
    import concourse.mybir as mybir
    import concourse.tile as tile

    f32 = mybir.dt.float32
    f32r = {
        "f32r": mybir.dt.float32r,
        "fp16": mybir.dt.float16,
        "bf16": mybir.dt.bfloat16,
    }[conv_dtype or CONV_DTYPE]
    if out_fp16 is None:
        out_fp16 = OUT_FP16
    fout = mybir.dt.float16 if out_fp16 else f32
    if drain is None:
        drain = DRAIN
    if order is None:
        order = ORDER
    if store_chunks is None:
        store_chunks = STORE_CHUNKS
    if dma_rings is None:
        dma_rings = DMA_RINGS
    if xbufs is None:
        xbufs = XBUFS
    if drain_width is None:
        drain_width = DRAIN_WIDTH
    assert NMT % drain_width == 0
    assert drain_width == 1 or order == "sgt"

    nc = bacc.Bacc(None, target_bir_lowering=False, name="ctxconv1d")

    xq_d = nc.dram_tensor("xq", [NB, GROUPS, 128, MPAD], f32r, kind="ExternalInput")
    wq_d = nc.dram_tensor("wq", [128, GROUPS, NSHIFT, 128], f32r, kind="ExternalInput")
    cwT2_d = nc.dram_tensor("cwT2", [C_DIM, GROUPS, 128], f32, kind="ExternalInput")
    cT_d = nc.dram_tensor("cT", [C_DIM, NB], f32, kind="ExternalInput")
    biasT2_d = nc.dram_tensor("biasT2", [128, GROUPS], f32, kind="ExternalInput")
    outq_d = nc.dram_tensor("outq", [NB, GROUPS, 128, M], fout, kind="ExternalOutput")

    with tile.TileContext(nc) as tc:
        with (
            tc.tile_pool(name="consts", bufs=1) as consts,
            tc.tile_pool(name="xpool", bufs=xbufs) as xpool,
            tc.tile_pool(name="opool", bufs=8) as opool,
            tc.tile_pool(name="ppool", bufs=8 // drain_width, space="PSUM") as ppool,
        ):
            wq_sb = consts.tile([128, GROUPS, NSHIFT, 128], f32r)
            cwT2_sb = consts.tile([C_DIM, GROUPS, 128], f32)
            cT_sb = consts.tile([C_DIM, NB], f32)
            biasT2_sb = consts.tile([128, GROUPS], f32)
            ctx2_sb = consts.tile([128, GROUPS, NB], f32)

            nc.sync.dma_start(wq_sb[:], wq_d[:])
            nc.sync.dma_start(cwT2_sb[:], cwT2_d[:])
            nc.sync.dma_start(cT_sb[:], cT_d[:])
            nc.sync.dma_start(biasT2_sb[:], biasT2_d[:])

            # ctx2[(o,d), g, n] = sum_dim c_weight[g*64+o, dim] * c[n, dim] + bias
            # (columns duplicated across the two output parities d)
            for g in range(GROUPS):
                # Same [128, MT] shape as the conv tiles so the pool keeps a
                # single tag (bufs are reserved per distinct shape).
                ctx_ps = ppool.tile([128, drain_width * MT], f32, tag="ps")
                nc.tensor.matmul(
                    ctx_ps[:, 0:NB], cwT2_sb[:, g, :], cT_sb[:], start=True, stop=True
                )
                nc.vector.tensor_scalar_add(
                    ctx2_sb[:, g, :], ctx_ps[:, 0:NB], biasT2_sb[:, g:g + 1]
                )

            use_big = BIG_TILES if big_tiles is None else big_tiles

            # With drain="split" the ACT queue does drains, so DMA issuance
            # defaults to the otherwise-idle Pool (gpsimd, SWDGE) queue;
            # "sp_act" keeps both fast HWDGE rings at the cost of sharing
            # the ACT queue between drains and DMA issuance.
            if dma_rings == "sp_act":
                dma_engines = (nc.sync, nc.scalar)
            elif drain == "split":
                dma_engines = (nc.sync, nc.gpsimd)
            else:
                dma_engines = (nc.sync, nc.scalar)

            def drain_op(k, out_ap, ps_ap, bias_ap):
                if drain == "split" and k % 2 == 1:
                    nc.scalar.activation(
                        out_ap, ps_ap,
                        mybir.ActivationFunctionType.Identity,
                        bias=bias_ap,
                    )
                else:
                    nc.vector.tensor_scalar_add(out_ap, ps_ap, bias_ap)

            def body():
                idx = 0
                dk = 0
                for n in range(NB):
                    for g in range(GROUPS):
                        ld = dma_engines[idx % 2]
                        st = dma_engines[(idx + 1) % 2]
                        idx += 1
                        if use_big and order == "sgt":
                            x_t = xpool.tile([128, MPAD], f32r)
                            ld.dma_start(x_t[:], xq_d[n, g, :, :])
                            o_t = opool.tile([128, M], fout)
                            dw = drain_width
                            pss = [
                                ppool.tile([128, dw * MT], f32, name=f"ps{j}",
                                           tag="ps")
                                for j in range(NMT // dw)
                            ]
                            for s in range(NSHIFT):
                                for t in range(NMT):
                                    off = (t % dw) * MT
                                    nc.tensor.matmul(
                                        pss[t // dw][:, off:off + MT],
                                        wq_sb[:, g, s, :],
                                        x_t[:, t * MT + s:t * MT + s + MT],
                                        start=(s == 0),
                                        stop=(s == NSHIFT - 1),
                                    )
                            for j in range(NMT // dw):
                                drain_op(
                                    dk, o_t[:, j * dw * MT:(j + 1) * dw * MT],
                                    pss[j][:], ctx2_sb[:, g, n:n + 1],
                                )
                                dk += 1
                            ch = M // store_chunks
                            for c in range(store_chunks):
                                st.dma_start(
                                    outq_d[n, g, :, c * ch:(c + 1) * ch],
                                    o_t[:, c * ch:(c + 1) * ch],
                                )
                        elif use_big:
                            x_t = xpool.tile([128, MPAD], f32r)
                            ld.dma_start(x_t[:], xq_d[n, g, :, :])
                            o_t = opool.tile([128, M], fout)
                            for t in range(NMT):
                                ps = ppool.tile([128, MT], f32, tag="ps")
                                for s in range(NSHIFT):
                                    nc.tensor.matmul(
                                        ps[:],
                                        wq_sb[:, g, s, :],
                                        x_t[:, t * MT + s:t * MT + s + MT],
                                        start=(s == 0),
                                        stop=(s == NSHIFT - 1),
                                    )
                                drain_op(
                                    dk, o_t[:, t * MT:(t + 1) * MT], ps[:],
                                    ctx2_sb[:, g, n:n + 1],
                                )
                                dk += 1
                            st.dma_start(outq_d[n, g, :, :], o_t[:])
                        else:
                            for t in range(NMT):
                                ld = dma_engines[idx % 2]
                                st = dma_engines[(idx + 1) % 2]
                                idx += 1
                                x_t = xpool.tile([128, MT + HALO], f32r)
                                ld.dma_start(
                                    x_t[:], xq_d[n, g, :, t * MT:t * MT + MT + HALO]
                                )
                                ps = ppool.tile([128, MT], f32, tag="ps")
                                for s in range(NSHIFT):
                                    nc.tensor.matmul(
                                        ps[:],
                                        wq_sb[:, g, s, :],
                                        x_t[:, s:s + MT],
                                        start=(s == 0),
                                        stop=(s == NSHIFT - 1),
                                    )
                                o_t = opool.tile([128, MT], fout)
                                drain_op(dk, o_t[:], ps[:], ctx2_sb[:, g, n:n + 1])
                                dk += 1
                                st.dma_start(
                                    outq_d[n, g, :, t * MT:(t + 1) * MT], o_t[:]
                                )

            if repeats == 1:
                body()
            else:
                # Big body (>256 insts/engine): arm back-edge prefetch so
                # repeat-loop timing isn't polluted by IRAM refetch stalls.
                with tc.For_i(
                    0, repeats, 1,
                    hint_engines=(
                        mybir.EngineType.PE,
                        mybir.EngineType.SP,
                        mybir.EngineType.Activation,
                        mybir.EngineType.DVE,
                        mybir.EngineType.Pool,
                    ),
                ):
                    body()

    nc.compile()
    return nc


def _get_program():
    if "nc" not in _prog_cache:
        _prog_cache["nc"] = _build_program()
    return _prog_cache["nc"]


def _conv_np_dtype(conv_dtype: str | None = None):
    import ml_dtypes

    return {
        "f32r": np.float32,
        "fp16": np.float16,
        "bf16": ml_dtypes.bfloat16,
    }[conv_dtype or CONV_DTYPE]


def _host_prep(x, c, weight, c_weight, bias, conv_dtype: str | None = None):
    # Phase-split padded x: xq[n, g, ph*64 + i, j] = xpad[n, g*64+i, 2j+ph]
    xp = np.zeros((N, C_IN, L + 2 * PAD), np.float32)
    xp[:, :, PAD:PAD + L] = x
    # (N, 4, 64, MPAD, 2) -> (N, 4, 2, 64, MPAD)
    xq = np.ascontiguousarray(
        xp.reshape(N, GROUPS, 64, MPAD, 2).transpose(0, 1, 4, 2, 3)
    ).reshape(N, GROUPS, 128, MPAD)

    # Polyphase stationary operands.
    wq = np.zeros((128, GROUPS, NSHIFT, 128), np.float32)
    for g in range(GROUPS):
        wg = weight[g * 64:(g + 1) * 64]          # (64 o, 64 i, K)
        for s in range(NSHIFT):
            wq[0:64, g, s, 0:64] = wg[:, :, 2 * s].T
            if 2 * s + 1 < K:
                wq[64:128, g, s, 0:64] = wg[:, :, 2 * s + 1].T
            if 2 * s - 1 >= 0:
                wq[0:64, g, s, 64:128] = wg[:, :, 2 * s - 1].T
            wq[64:128, g, s, 64:128] = wg[:, :, 2 * s].T

    # cwT2[d, g, 64*delta + o] = c_weight[g*64 + o, d]  (parity-duplicated)
    cwT2 = np.zeros((C_DIM, GROUPS, 128), np.float32)
    cw = c_weight.reshape(GROUPS, 64, C_DIM)
    for g in range(GROUPS):
        cwT2[:, g, 0:64] = cw[g].T
        cwT2[:, g, 64:128] = cw[g].T

    biasT2 = np.zeros((128, GROUPS), np.float32)
    b = bias.reshape(GROUPS, 64)
    biasT2[0:64] = b.T
    biasT2[64:128] = b.T

    cT = np.ascontiguousarray(c.T)  # (128, 32)

    npdt = _conv_np_dtype(conv_dtype)
    xq = xq.astype(npdt, copy=False)
    wq = wq.astype(npdt, copy=False)
    return xq, wq, cwT2, cT, biasT2


def kernel(x, c, weight, c_weight, bias):
    global LAST_RESULT
    from concourse.bass_utils import run_bass_kernel_spmd

    x = np.asarray(x, dtype=np.float32)
    c = np.asarray(c, dtype=np.float32)
    weight = np.asarray(weight, dtype=np.float32)
    c_weight = np.asarray(c_weight, dtype=np.float32)
    bias = np.asarray(bias, dtype=np.float32)

    xq, wq, cwT2, cT, biasT2 = _host_prep(x, c, weight, c_weight, bias)

    in_maps = []
    for i in range(NCORES):
        in_maps.append({
            "xq": np.ascontiguousarray(xq[i * NB:(i + 1) * NB]),
            "wq": wq,
            "cwT2": cwT2,
            "cT": np.ascontiguousarray(cT[:, i * NB:(i + 1) * NB]),
            "biasT2": biasT2,
        })

    nc = _get_program()
    res = run_bass_kernel_spmd(nc, in_maps, core_ids=list(range(NCORES)), **RUN_KWARGS)
    LAST_RESULT = res

    outq = np.concatenate([r["outq"] for r in res.results], axis=0)  # (N,4,128,M)
    # y[n, g*64+o, 2m+d] = outq[n, g, 64d+o, m]
    y = np.ascontiguousarray(
        outq.astype(np.float32, copy=False)
        .reshape(N, GROUPS, 2, 64, M)
        .transpose(0, 1, 3, 4, 2)
    ).reshape(N, C_OUT, L)
    return np.ascontiguousarray(y)

